# revision 1
# baseline (speedup 1.0000x reference)
"""Trainium2 Bass kernel for the masked-softmax attention module.

Computation (per batch row b):
    m      = lrelu(values[b] @ Wv.T + bv) + lrelu(query[b] @ Wq.T + bq)   [L, A]
    logit  = lrelu(tanh(m) @ Wo.T + bo)                                    [L]
    logit  = where(mask[b] == 0, -1e-9, logit)
    prob   = softmax(logit)
    out[b] = prob @ values[b]                                              [D]

Sharding: data-parallel over batch, 4 batches per core on 8 NeuronCores.
All FLOPs run on device; the host only reshapes/casts/shards inputs.

Device dataflow per (batch, l-chunk of 512):
    for each a-tile j (16 of 128):
        PSUM[a=128, l=512]  = sum_k WvT[k,j] (stationary) @ valuesT[k, lchunk]
        s1   = ACT Lrelu(PSUM + bv[j])            (per-partition bias)
        tanh = ACT Tanh(s1 + qp[j, b]) -> bf16    (qp = lrelu(q proj), precomputed on device)
        logit PSUM[1, 512] += Wo[j] (stationary) @ tanh      (accumulated over j)
    masked softmax on [1, L] (DVE/ACT), prob -> bf16, PE-transpose to [l, 1] tiles,
    out[b] PSUM[1, 512] += probT (stationary) @ values_nat[ltile, dchunk]
"""

import os
import sys

if "/opt/trn_rl_repo" not in sys.path:
    sys.path.insert(0, "/opt/trn_rl_repo")

import numpy as np
import ml_dtypes

from contextlib import ExitStack

import concourse.bass as bass
import concourse.tile as tile
from concourse import bacc, mybir
from concourse import bass_utils

BF = ml_dtypes.bfloat16
F32d = mybir.dt.float32
BF16d = mybir.dt.bfloat16
AF = mybir.ActivationFunctionType

NCORES = 8
B, L, D, A = 32, 1024, 2048, 2048
BL = B // NCORES          # batches per core
KD = D // 128             # d tiles
KA = A // 128             # a tiles
JL = L // 128             # l tiles
NL = 512                  # l chunk (PSUM free-dim limit for f32)
NCH = L // NL             # l chunks per batch
ALPHA = 0.01              # leaky relu slope


def build_graph(lrelu_mode="act"):
    """Build the per-core Bass graph (identical on all cores)."""
    nc = bacc.Bacc("TRN2", target_bir_lowering=False, debug=False)

    vt = nc.dram_tensor("vt", [BL, 128, NCH, KD, NL], BF16d, kind="ExternalInput")
    vn = nc.dram_tensor("vn", [BL, 128, JL, D], BF16d, kind="ExternalInput")
    wv = nc.dram_tensor("wv", [128, KA, KD, 128], BF16d, kind="ExternalInput")
    wq = nc.dram_tensor("wq", [128, KA, KD, 128], BF16d, kind="ExternalInput")
    qt = nc.dram_tensor("qt", [128, KD, BL], BF16d, kind="ExternalInput")
    wo = nc.dram_tensor("wo", [128, KA], BF16d, kind="ExternalInput")
    bvt = nc.dram_tensor("bvt", [128, KA], F32d, kind="ExternalInput")
    bqt = nc.dram_tensor("bqt", [128, KA], F32d, kind="ExternalInput")
    bo = nc.dram_tensor("bo", [1, 1], F32d, kind="ExternalInput")
    mf = nc.dram_tensor("mf", [1, BL * L], F32d, kind="ExternalInput")
    madd = nc.dram_tensor("madd", [1, BL * L], F32d, kind="ExternalInput")
    id4d = nc.dram_tensor("id4", [JL, JL], F32d, kind="ExternalInput")
    out = nc.dram_tensor("out", [BL, D], F32d, kind="ExternalOutput")

    def act_lrelu(out_ap, in_ap, bias_ap, pool, shape):
        if lrelu_mode == "act":
            nc.scalar.activation(out_ap, in_ap, AF.Lrelu, bias=bias_ap, alpha=ALPHA)
        else:
            # DVE fallback: lrelu(x + b) = max(x + b, ALPHA * (x + b))
            t1 = pool.tile(shape, F32d, tag="lr1")
            t2 = pool.tile(shape, F32d, tag="lr2")
            nc.vector.tensor_scalar(t1[:], in_ap, bias_ap, None, mybir.AluOpType.add)
            nc.vector.tensor_scalar(
                t2[:], in_ap, bias_ap, ALPHA, mybir.AluOpType.add, mybir.AluOpType.mult
            )
            nc.vector.tensor_max(out_ap, t1[:], t2[:])

    with tile.TileContext(nc) as tc, ExitStack() as ctx:
        const = ctx.enter_context(tc.tile_pool(name="const", bufs=1))
        wvp = ctx.enter_context(tc.tile_pool(name="wvp", bufs=1))

        # One FIFO HWDGE ring (sync) carries all latency-ordered loads in exact
        # need-order; only bulk vn (needed a full batch later) rides the
        # scalar ring in parallel.
        qts_sb = const.tile([128, KD, BL], BF16d)
        nc.sync.dma_start(qts_sb[:], qt.ap()[:])
        bq_sb = const.tile([128, KA], F32d)
        nc.sync.dma_start(bq_sb[:], bqt.ap()[:])
        id8 = const.tile([JL, JL], F32d)
        nc.scalar.dma_start(id8[:], id4d.ap()[:])
        qp_sb = const.tile([128, KA, BL], F32d)
        # allocated here, DMA'd after the first vt chunk (see main loop)
        wo_sb = const.tile([128, KA], BF16d)
        bv_sb = const.tile([128, KA], F32d)
        bo_sb = const.tile([1, 1], F32d)
        mf_sb = const.tile([1, BL * L], F32d)
        madd_sb = const.tile([1, BL * L], F32d)

        # wv is laid out a-tile-major: GEMM group j only needs its own 512KB
        # chunk, so chunks stream just-in-time, interleaved with wq below.
        wv_sb = wvp.tile([128, KA, KD, 128], BF16d)
        wv_loaded = set()

        def fetch_wv(j):
            if j < KA and j not in wv_loaded:
                nc.sync.dma_start(wv_sb[:, j, :, :], wv.ap()[:, j, :, :])
                wv_loaded.add(j)

        # q-projection is interleaved into batch 0 / chunk 0 of the main loop
        # (one group per GEMM group) so its wq DMA demand spreads out and the
        # PE never sits idle waiting for the projection phase.
        wqp = ctx.enter_context(tc.tile_pool(name="wqp", bufs=3))
        psqp = ctx.enter_context(tc.tile_pool(name="psq", bufs=1, space="PSUM"))
        lrq = ctx.enter_context(tc.tile_pool(name="lrq", bufs=2))
        wq_tiles = {}

        def fetch_wq(t):
            if t < KA and t not in wq_tiles:
                wq_t = wqp.tile([128, KD, 128], BF16d)
                nc.sync.dma_start(wq_t[:], wq.ap()[:, t, :, :])
                wq_tiles[t] = wq_t

        def qproj_group(t):
            wq_t = wq_tiles.pop(t)
            psq = psqp.tile([128, BL], F32d)
            for k in range(KD):
                nc.tensor.matmul(
                    psq[:], lhsT=wq_t[:, k, :], rhs=qts_sb[:, k, :],
                    start=(k == 0), stop=(k == KD - 1),
                )
            act_lrelu(qp_sb[:, t, :], psq[:], bq_sb[:, t : t + 1], lrq, [128, BL])

        # PE warmup: dummy matmuls on zeroed tiles while the first DMAs land,
        # so the HAM clock gate is released before real work starts.
        wu_l = const.tile([128, 128], BF16d)
        nc.vector.memset(wu_l[:], 0.0)
        wu_ps = psqp.tile([128, 128], F32d, tag="psq")
        for i in range(32):
            nc.tensor.matmul(wu_ps[:], lhsT=wu_l[:], rhs=wu_l[:], start=(i == 0), stop=(i == 31))

        # ---- main loop ----
        vtp = ctx.enter_context(tc.tile_pool(name="vtp", bufs=2))
        vnp = ctx.enter_context(tc.tile_pool(name="vnp", bufs=1))
        s1p = ctx.enter_context(tc.tile_pool(name="s1p", bufs=2))
        thp = ctx.enter_context(tc.tile_pool(name="thp", bufs=3))
        smp = ctx.enter_context(tc.tile_pool(name="smp", bufs=1))
        outp = ctx.enter_context(tc.tile_pool(name="outp", bufs=1))
        psm = ctx.enter_context(tc.tile_pool(name="psm", bufs=2, space="PSUM"))
        psl = ctx.enter_context(tc.tile_pool(name="psl", bufs=2, space="PSUM"))
        pst = ctx.enter_context(tc.tile_pool(name="pst", bufs=1, space="PSUM"))
        pso = ctx.enter_context(tc.tile_pool(name="pso", bufs=2, space="PSUM"))

        for b in range(BL):
            vn_b = None
            logit_sb = smp.tile([1, L], F32d, tag="logit")
            for c in range(NCH):
                if c == NCH - 1:
                    # natural-orientation values load, deferred past the
                    # DMA-congested first chunk (only needed at batch end)
                    vn_b = vnp.tile([128, JL, D], BF16d)
                    nc.scalar.dma_start(vn_b[:], vn.ap()[b, :, :, :])
                if b == 0 and c == 0:
                    fetch_wq(0)
                vt_c = vtp.tile([128, KD, NL], BF16d)
                nc.sync.dma_start(vt_c[:], vt.ap()[b, :, c, :, :])
                if b == 0 and c == 0:
                    fetch_wv(0)
                    fetch_wq(1)
                    nc.scalar.dma_start(wo_sb[:], wo.ap()[:])
                    nc.scalar.dma_start(bv_sb[:], bvt.ap()[:])
                    nc.scalar.dma_start(bo_sb[:], bo.ap()[:])
                    nc.scalar.dma_start(mf_sb[:], mf.ap()[:])
                    nc.scalar.dma_start(madd_sb[:], madd.ap()[:])
                ps_l = psl.tile([1, NL], F32d)
                pending = None
                for j in range(KA):
                    if b == 0 and c == 0:
                        qproj_group(j)
                        fetch_wq(j + 2)
                        fetch_wv(j + 1)
                        fetch_wv(j + 2)
                    ps_m = psm.tile([128, NL], F32d)
                    for k in range(KD):
                        nc.tensor.matmul(
                            ps_m[:],
                            lhsT=wv_sb[:, j, k, :],
                            rhs=vt_c[:, k, :],
                            start=(k == 0),
                            stop=(k == KD - 1),
                        )
                    s1 = s1p.tile([128, NL], F32d)
                    act_lrelu(s1[:], ps_m[:], bv_sb[:, j : j + 1], s1p, [128, NL])
                    th = thp.tile([128, NL], BF16d)
                    nc.scalar.activation(th[:], s1[:], AF.Tanh, bias=qp_sb[:, j, b : b + 1])
                    if pending is not None:
                        pj, pth = pending
                        nc.tensor.matmul(
                            ps_l[:], lhsT=wo_sb[:, pj : pj + 1], rhs=pth[:],
                            start=(pj == 0), stop=False,
                        )
                    pending = (j, th)
                pj, pth = pending
                nc.tensor.matmul(
                    ps_l[:], lhsT=wo_sb[:, pj : pj + 1], rhs=pth[:], start=False, stop=True
                )
                lsl = logit_sb[:, NL * c : NL * c + NL]
                if lrelu_mode == "act":
                    nc.scalar.activation(
                        lsl, ps_l[:], AF.Lrelu, bias=bo_sb[0:1, 0:1], alpha=ALPHA
                    )
                else:
                    act_lrelu(lsl, ps_l[:], bo_sb[0:1, 0:1], smp, [1, NL])
                # apply the mask per chunk (off the end-of-batch critical path)
                o = b * L + NL * c
                nc.vector.tensor_mul(lsl, lsl, mf_sb[:, o : o + NL])
                nc.vector.tensor_add(lsl, lsl, madd_sb[:, o : o + NL])

            # ---- softmax (no max-subtract: logits are lrelu-bounded, exp stays
            # well inside f32 range; identical ratios to the reference) ----
            p_f = smp.tile([1, L], F32d, tag="pf")
            ssum = smp.tile([1, 1], F32d, tag="ss")
            nc.scalar.activation(p_f[:], logit_sb[:], AF.Exp, accum_out=ssum[:])
            rs = smp.tile([1, 1], F32d, tag="rs")
            nc.vector.reciprocal(rs[:], ssum[:])
            # cross-partition reshape [1, L] -> [JL, 128] (tiny DMA), then one
            # PE transpose to [128, JL]
            p8 = smp.tile([JL, 128], F32d, tag="p8")
            nc.scalar.dma_start(p8[:], p_f[:])
            ps_t = pst.tile([128, JL], F32d)
            nc.tensor.transpose(ps_t[:], p8[:], id8[:])
            pT = smp.tile([128, JL], BF16d, tag="pT")
            nc.vector.tensor_copy(pT[:], ps_t[:])

            # ---- out[b] = (p @ values) / sum  (1/sum folded into the copy) ----
            out_sb = outp.tile([1, D], F32d)
            for dc in range(4):
                ps_o = pso.tile([1, 512], F32d)
                for t in range(JL):
                    nc.tensor.matmul(
                        ps_o[:], lhsT=pT[:, t : t + 1],
                        rhs=vn_b[:, t, 512 * dc : 512 * dc + 512],
                        start=(t == 0), stop=(t == JL - 1),
                    )
                osl = out_sb[:, 512 * dc : 512 * dc + 512]
                nc.vector.tensor_scalar_mul(osl, ps_o[:], rs[0:1, 0:1])
                nc.sync.dma_start(out.ap()[b : b + 1, 512 * dc : 512 * dc + 512], osl)

    nc.compile()
    return nc


def prep_inputs(query, values, mask, Wq, bq, Wv, bv, Wo, bo):
    """Host-side shard + layout prep. Returns list of 8 in_maps."""
    Wv32 = np.ascontiguousarray(Wv, np.float32)
    Wq32 = np.ascontiguousarray(Wq, np.float32)
    # wv[p, j, k, i] = Wv[128j+i, 128k+p]  (WvT, a-tile-major chunks)
    wv_t = np.ascontiguousarray(
        Wv32.reshape(KA, 128, KD, 128).transpose(3, 0, 2, 1)
    ).astype(BF)
    # wq[p, t, k, i] = Wq[128t+i, 128k+p]  (WqT, a-tile-major chunks)
    wq_t = np.ascontiguousarray(
        Wq32.reshape(KA, 128, KD, 128).transpose(3, 0, 2, 1)
    ).astype(BF)
    wo_t = np.ascontiguousarray(Wo.reshape(KA, 128).T).astype(BF)
    bv_t = np.ascontiguousarray(bv.reshape(KA, 128).T).astype(np.float32)
    bq_t = np.ascontiguousarray(bq.reshape(KA, 128).T).astype(np.float32)
    bo_r = np.asarray(bo, np.float32).reshape(1, 1)

    in_maps = []
    for i in range(NCORES):
        sl = slice(BL * i, BL * (i + 1))
        v = np.asarray(values[sl], np.float32)
        # vt[b, p, c, k, l] = values[b, 512c+l, 128k+p]
        vt_i = np.ascontiguousarray(
            v.transpose(0, 2, 1)
            .reshape(BL, KD, 128, NCH, NL)
            .transpose(0, 2, 3, 1, 4)
        ).astype(BF)
        # vn[b, p, j, d] = values[b, 128j+p, d]
        vn_i = np.ascontiguousarray(
            v.reshape(BL, JL, 128, D).transpose(0, 2, 1, 3)
        ).astype(BF)
        # qt[p, k, b] = query[b, 128k+p]
        qt_i = np.ascontiguousarray(
            np.asarray(query[sl], np.float32).T.reshape(KD, 128, BL).transpose(1, 0, 2)
        ).astype(BF)
        m = np.asarray(mask[sl])
        mf_i = (m != 0).astype(np.float32).reshape(1, BL * L)
        madd_i = ((m == 0).astype(np.float32) * np.float32(-1e-9)).reshape(1, BL * L)
        in_maps.append(
            {
                "vt": vt_i, "vn": vn_i, "wv": wv_t, "wq": wq_t, "qt": qt_i,
                "wo": wo_t, "bvt": bv_t, "bqt": bq_t, "bo": bo_r,
                "mf": mf_i, "madd": madd_i, "id4": np.eye(JL, dtype=np.float32),
            }
        )
    return in_maps


_NC_CACHE = {}


def get_graph(lrelu_mode="act"):
    if lrelu_mode not in _NC_CACHE:
        _NC_CACHE[lrelu_mode] = build_graph(lrelu_mode)
    return _NC_CACHE[lrelu_mode]


def run(inputs, trace=False, lrelu_mode="act"):
    nc = get_graph(lrelu_mode)
    in_maps = prep_inputs(**inputs)
    res = bass_utils.run_bass_kernel_spmd(
        nc, in_maps, core_ids=list(range(NCORES)), trace=trace
    )
    out = np.concatenate([res.results[i]["out"] for i in range(NCORES)], axis=0)
    return out.astype(np.float32), res


def kernel(**inputs):
    out, _ = run(inputs, trace=False)
    return out



# revision 2
# speedup vs baseline: 1.5150x; 1.5150x over previous
"""Trainium2 Bass kernel for the masked-softmax attention module.

Computation (per batch row b):
    m      = lrelu(values[b] @ Wv.T + bv) + lrelu(query[b] @ Wq.T + bq)   [L, A]
    logit  = lrelu(tanh(m) @ Wo.T + bo)                                    [L]
    logit  = where(mask[b] == 0, -1e-9, logit)
    prob   = softmax(logit)
    out[b] = prob @ values[b]                                              [D]

Sparsity: positions with mask==0 get logit = -1e-9, so their softmax
weight is exactly exp(-1e-9) == 1.0f regardless of the expensive
pipeline. Host-side we PERMUTE each batch's L dim so mask==1 positions
come first (n1 of them), and only compute the m/tanh/Wo pipeline for
the first N >= max_b(n1) positions. Logits at [n1, N) are masked to
-1e-9 (exactly as the reference masks them) and [N, L) are memset to 0
(exp(0) == exp(-1e-9) == 1.0f). The softmax + out GEMM then run over
the full permuted L — numerically identical to the dense reference.

Main GEMM runs in fp8 (e4m3): values cast directly (absmax ~5.4 << 240),
Wv pre-scaled by 2^10 so its entries are normal-range; the 2^-10 unscale
is folded into the lrelu ACT's scale input (exact, power of two).
DoubleRow perf mode processes 2 k-tiles per matmul at 0.5 cycles/row.

Sharding: data-parallel over batch, 4 batches per core on 8 NeuronCores.
"""

import os
import sys

if "/opt/trn_rl_repo" not in sys.path:
    sys.path.insert(0, "/opt/trn_rl_repo")

import numpy as np
import ml_dtypes

from contextlib import ExitStack

import concourse.bass as bass
import concourse.tile as tile
from concourse import bacc, mybir
from concourse import bass_utils

BF = ml_dtypes.bfloat16
E4 = ml_dtypes.float8_e4m3
F32d = mybir.dt.float32
BF16d = mybir.dt.bfloat16
FP8d = mybir.dt.float8e4
AF = mybir.ActivationFunctionType
DR = mybir.MatmulPerfMode.DoubleRow

NCORES = 8
B, L, D, A = 32, 1024, 2048, 2048
BL = B // NCORES          # batches per core
KD = D // 128             # d tiles
KA = A // 128             # a tiles
JL = L // 128             # l tiles
ALPHA = 0.01              # leaky relu slope
WV_SCALE = 1024.0         # host premultiplier on Wv for fp8 dynamic range


def build_graph(N, mm="fp8"):
    """Build the per-core Bass graph (identical on all cores).

    N: padded count of computed positions per batch (even).
    mm: "fp8" (DoubleRow e4m3 main GEMM) or "bf16".
    """
    nc = bacc.Bacc("TRN2", target_bir_lowering=False, debug=False)
    NL1 = N // 2
    chunks = [(0, NL1), (NL1, NL1)]
    vdt, vnp_dt = (FP8d, E4) if mm == "fp8" else (BF16d, BF)

    vt = nc.dram_tensor("vt", [BL, 128, KD, N], vdt, kind="ExternalInput")
    vn = nc.dram_tensor("vn", [BL, 128, JL, D], BF16d, kind="ExternalInput")
    wv = nc.dram_tensor("wv", [128, KA, KD, 128], vdt, kind="ExternalInput")
    wq = nc.dram_tensor("wq", [128, KA, KD, 128], BF16d, kind="ExternalInput")
    qt = nc.dram_tensor("qt", [128, KD, BL], BF16d, kind="ExternalInput")
    wo = nc.dram_tensor("wo", [128, KA], BF16d, kind="ExternalInput")
    bvt = nc.dram_tensor("bvt", [128, KA], F32d, kind="ExternalInput")
    bqt = nc.dram_tensor("bqt", [128, KA], F32d, kind="ExternalInput")
    bo = nc.dram_tensor("bo", [1, 1], F32d, kind="ExternalInput")
    mf = nc.dram_tensor("mf", [1, BL * N], F32d, kind="ExternalInput")
    madd = nc.dram_tensor("madd", [1, BL * N], F32d, kind="ExternalInput")
    id4d = nc.dram_tensor("id4", [JL, JL], F32d, kind="ExternalInput")
    out = nc.dram_tensor("out", [BL, D], F32d, kind="ExternalOutput")

    lr_scale = 1.0 / WV_SCALE if mm == "fp8" else 1.0

    with tile.TileContext(nc) as tc, ExitStack() as ctx:
        const = ctx.enter_context(tc.tile_pool(name="const", bufs=1))
        wvp = ctx.enter_context(tc.tile_pool(name="wvp", bufs=1))

        # One FIFO HWDGE ring (sync) carries all latency-ordered loads in exact
        # need-order; only bulk vn (needed a full batch later) rides the
        # scalar ring in parallel.
        qts_sb = const.tile([128, KD, BL], BF16d)
        nc.sync.dma_start(qts_sb[:], qt.ap()[:])
        bq_sb = const.tile([128, KA], F32d)
        nc.sync.dma_start(bq_sb[:], bqt.ap()[:])
        id8 = const.tile([JL, JL], F32d)
        nc.scalar.dma_start(id8[:], id4d.ap()[:])
        qp_sb = const.tile([128, KA, BL], F32d)
        # allocated here, DMA'd after the first vt chunk (see main loop)
        wo_sb = const.tile([128, KA], BF16d)
        bv_sb = const.tile([128, KA], F32d)
        bo_sb = const.tile([1, 1], F32d)
        mf_sb = const.tile([1, BL * N], F32d)
        madd_sb = const.tile([1, BL * N], F32d)

        # wv is laid out a-tile-major: GEMM group j only needs its own chunk,
        # so chunks stream just-in-time, interleaved with wq below.
        wv_sb = wvp.tile([128, KA, KD, 128], vdt)
        wv_loaded = set()

        def fetch_wv(j):
            if j < KA and j not in wv_loaded:
                nc.sync.dma_start(wv_sb[:, j, :, :], wv.ap()[:, j, :, :])
                wv_loaded.add(j)

        # q-projection is interleaved into batch 0 / chunk 0 of the main loop
        # (one group per GEMM group) so its wq DMA demand spreads out and the
        # PE never sits idle waiting for the projection phase.
        wqp = ctx.enter_context(tc.tile_pool(name="wqp", bufs=3))
        psqp = ctx.enter_context(tc.tile_pool(name="psq", bufs=1, space="PSUM"))
        wq_tiles = {}

        def fetch_wq(t):
            if t < KA and t not in wq_tiles:
                wq_t = wqp.tile([128, KD, 128], BF16d)
                nc.sync.dma_start(wq_t[:], wq.ap()[:, t, :, :])
                wq_tiles[t] = wq_t

        def qproj_group(t):
            wq_t = wq_tiles.pop(t)
            psq = psqp.tile([128, BL], F32d)
            for k in range(KD):
                nc.tensor.matmul(
                    psq[:], lhsT=wq_t[:, k, :], rhs=qts_sb[:, k, :],
                    start=(k == 0), stop=(k == KD - 1),
                )
            nc.scalar.activation(
                qp_sb[:, t, :], psq[:], AF.Lrelu, bias=bq_sb[:, t : t + 1],
                alpha=ALPHA,
            )

        # PE warmup: dummy matmuls on zeroed tiles while the first DMAs land,
        # so the HAM clock gate is released before real work starts.
        wu_l = const.tile([128, 128], BF16d)
        nc.vector.memset(wu_l[:], 0.0)
        wu_ps = psqp.tile([128, 128], F32d, tag="psq")
        for i in range(32):
            nc.tensor.matmul(wu_ps[:], lhsT=wu_l[:], rhs=wu_l[:], start=(i == 0), stop=(i == 31))

        # ---- main loop ----
        vtp = ctx.enter_context(tc.tile_pool(name="vtp", bufs=2))
        vnp = ctx.enter_context(tc.tile_pool(name="vnp", bufs=1))
        s1p = ctx.enter_context(tc.tile_pool(name="s1p", bufs=2))
        thp = ctx.enter_context(tc.tile_pool(name="thp", bufs=3))
        smp = ctx.enter_context(tc.tile_pool(name="smp", bufs=1))
        outp = ctx.enter_context(tc.tile_pool(name="outp", bufs=1))
        psm = ctx.enter_context(tc.tile_pool(name="psm", bufs=2, space="PSUM"))
        psl = ctx.enter_context(tc.tile_pool(name="psl", bufs=2, space="PSUM"))
        pst = ctx.enter_context(tc.tile_pool(name="pst", bufs=1, space="PSUM"))
        pso = ctx.enter_context(tc.tile_pool(name="pso", bufs=2, space="PSUM"))

        for b in range(BL):
            vn_b = None
            logit_sb = smp.tile([1, L], F32d, tag="logit")
            # tail [N, L) is never computed: weight must be exp(-1e-9) == 1.0f,
            # which equals exp(0), so zero-fill suffices.
            nc.vector.memset(logit_sb[:, N:L], 0.0)
            vt_b = vtp.tile([128, KD, N], vdt)
            nc.sync.dma_start(vt_b[:], vt.ap()[b, :, :, :])
            for ci, (off, nl) in enumerate(chunks):
                if ci == len(chunks) - 1:
                    # natural-orientation values load, deferred past the
                    # DMA-congested first chunk (only needed at batch end)
                    vn_b = vnp.tile([128, JL, D], BF16d)
                    nc.scalar.dma_start(vn_b[:], vn.ap()[b, :, :, :])
                if b == 0 and ci == 0:
                    fetch_wq(0)
                    fetch_wv(0)
                    fetch_wq(1)
                    nc.scalar.dma_start(wo_sb[:], wo.ap()[:])
                    nc.scalar.dma_start(bv_sb[:], bvt.ap()[:])
                    nc.scalar.dma_start(bo_sb[:], bo.ap()[:])
                    nc.scalar.dma_start(mf_sb[:], mf.ap()[:])
                    nc.scalar.dma_start(madd_sb[:], madd.ap()[:])
                ps_l = psl.tile([1, 512], F32d)
                pending = None
                for j in range(KA):
                    if b == 0 and ci == 0:
                        qproj_group(j)
                        fetch_wq(j + 2)
                        fetch_wv(j + 1)
                        fetch_wv(j + 2)
                    ps_m = psm.tile([128, 512], F32d)
                    if mm == "fp8":
                        for q in range(KD // 2):
                            nc.tensor.matmul(
                                ps_m[:, :nl],
                                lhsT=wv_sb[:, j, 2 * q : 2 * q + 2, :],
                                rhs=vt_b[:, 2 * q : 2 * q + 2, off : off + nl],
                                start=(q == 0),
                                stop=(q == KD // 2 - 1),
                                perf_mode=DR,
                            )
                    else:
                        for k in range(KD):
                            nc.tensor.matmul(
                                ps_m[:, :nl],
                                lhsT=wv_sb[:, j, k, :],
                                rhs=vt_b[:, k, off : off + nl],
                                start=(k == 0),
                                stop=(k == KD - 1),
                            )
                    s1 = s1p.tile([128, 512], F32d)
                    nc.scalar.activation(
                        s1[:, :nl], ps_m[:, :nl], AF.Lrelu,
                        bias=bv_sb[:, j : j + 1], scale=lr_scale, alpha=ALPHA,
                    )
                    th = thp.tile([128, 512], BF16d)
                    nc.scalar.activation(
                        th[:, :nl], s1[:, :nl], AF.Tanh, bias=qp_sb[:, j, b : b + 1]
                    )
                    if pending is not None:
                        pj, pth = pending
                        nc.tensor.matmul(
                            ps_l[:, :nl], lhsT=wo_sb[:, pj : pj + 1], rhs=pth[:, :nl],
                            start=(pj == 0), stop=False,
                        )
                    pending = (j, th)
                pj, pth = pending
                nc.tensor.matmul(
                    ps_l[:, :nl], lhsT=wo_sb[:, pj : pj + 1], rhs=pth[:, :nl],
                    start=False, stop=True,
                )
                lsl = logit_sb[:, off : off + nl]
                nc.scalar.activation(
                    lsl, ps_l[:, :nl], AF.Lrelu, bias=bo_sb[0:1, 0:1], alpha=ALPHA
                )
                # apply the mask per chunk (off the end-of-batch critical path)
                o = b * N + off
                nc.vector.tensor_mul(lsl, lsl, mf_sb[:, o : o + nl])
                nc.vector.tensor_add(lsl, lsl, madd_sb[:, o : o + nl])

            # ---- softmax (no max-subtract: logits are lrelu-bounded, exp stays
            # well inside f32 range; identical ratios to the reference) ----
            p_f = smp.tile([1, L], F32d, tag="pf")
            ssum = smp.tile([1, 1], F32d, tag="ss")
            nc.scalar.activation(p_f[:], logit_sb[:], AF.Exp, accum_out=ssum[:])
            rs = smp.tile([1, 1], F32d, tag="rs")
            nc.vector.reciprocal(rs[:], ssum[:])
            # cross-partition reshape [1, L] -> [JL, 128] (tiny DMA), then one
            # PE transpose to [128, JL]
            p8 = smp.tile([JL, 128], F32d, tag="p8")
            nc.scalar.dma_start(p8[:], p_f[:])
            ps_t = pst.tile([128, JL], F32d)
            nc.tensor.transpose(ps_t[:], p8[:], id8[:])
            pT = smp.tile([128, JL], BF16d, tag="pT")
            nc.vector.tensor_copy(pT[:], ps_t[:])

            # ---- out[b] = (p @ values) / sum  (1/sum folded into the copy) ----
            out_sb = outp.tile([1, D], F32d)
            for dc in range(4):
                ps_o = pso.tile([1, 512], F32d)
                for t in range(JL):
                    nc.tensor.matmul(
                        ps_o[:], lhsT=pT[:, t : t + 1],
                        rhs=vn_b[:, t, 512 * dc : 512 * dc + 512],
                        start=(t == 0), stop=(t == JL - 1),
                    )
                osl = out_sb[:, 512 * dc : 512 * dc + 512]
                nc.vector.tensor_scalar_mul(osl, ps_o[:], rs[0:1, 0:1])
                nc.sync.dma_start(out.ap()[b : b + 1, 512 * dc : 512 * dc + 512], osl)

    nc.compile()
    return nc


def pad_n(max_n1):
    """Computed-position count: even split into two equal chunks, each a
    multiple of 4 and >= 128 so the PE never stalls on LDWEIGHTS."""
    half = max(128, (max_n1 + 1) // 2)
    half = (half + 3) // 4 * 4
    return min(2 * half, L)


def prep_inputs(query, values, mask, Wq, bq, Wv, bv, Wo, bo, mm="fp8"):
    """Host-side shard + layout prep. Returns (N, list of 8 in_maps)."""
    mask = np.asarray(mask)
    n1s = (mask != 0).sum(axis=1)
    N = pad_n(int(n1s.max()))

    Wv32 = np.ascontiguousarray(Wv, np.float32)
    Wq32 = np.ascontiguousarray(Wq, np.float32)
    # wv[p, j, k, i] = Wv[128j+i, 128k+p]  (WvT, a-tile-major chunks)
    wv_t = np.ascontiguousarray(
        Wv32.reshape(KA, 128, KD, 128).transpose(3, 0, 2, 1)
    )
    if mm == "fp8":
        wv_t = (wv_t * np.float32(WV_SCALE)).astype(E4)
    else:
        wv_t = wv_t.astype(BF)
    # wq[p, t, k, i] = Wq[128t+i, 128k+p]  (WqT, a-tile-major chunks)
    wq_t = np.ascontiguousarray(
        Wq32.reshape(KA, 128, KD, 128).transpose(3, 0, 2, 1)
    ).astype(BF)
    wo_t = np.ascontiguousarray(Wo.reshape(KA, 128).T).astype(BF)
    bv_t = np.ascontiguousarray(bv.reshape(KA, 128).T).astype(np.float32)
    bq_t = np.ascontiguousarray(bq.reshape(KA, 128).T).astype(np.float32)
    bo_r = np.asarray(bo, np.float32).reshape(1, 1)

    in_maps = []
    for i in range(NCORES):
        sl = slice(BL * i, BL * (i + 1))
        v = np.asarray(values[sl], np.float32)
        m = np.asarray(mask[sl])
        # permute each batch's L dim: mask!=0 positions first
        vp = np.empty_like(v)
        mp = np.empty_like(m)
        for bb in range(BL):
            perm = np.concatenate(
                [np.flatnonzero(m[bb] != 0), np.flatnonzero(m[bb] == 0)]
            )
            vp[bb] = v[bb, perm]
            mp[bb] = m[bb, perm]
        # vt[b, p, k, l] = vp[b, l, 128k+p] for l < N
        vt_i = np.ascontiguousarray(
            vp[:, :N, :].reshape(BL, N, KD, 128).transpose(0, 3, 2, 1)
        )
        vt_i = vt_i.astype(E4) if mm == "fp8" else vt_i.astype(BF)
        # vn[b, p, j, d] = vp[b, 128j+p, d]
        vn_i = np.ascontiguousarray(
            vp.reshape(BL, JL, 128, D).transpose(0, 2, 1, 3)
        ).astype(BF)
        # qt[p, k, b] = query[b, 128k+p]
        qt_i = np.ascontiguousarray(
            np.asarray(query[sl], np.float32).T.reshape(KD, 128, BL).transpose(1, 0, 2)
        ).astype(BF)
        mf_i = (mp[:, :N] != 0).astype(np.float32).reshape(1, BL * N)
        madd_i = ((mp[:, :N] == 0).astype(np.float32) * np.float32(-1e-9)).reshape(
            1, BL * N
        )
        in_maps.append(
            {
                "vt": vt_i, "vn": vn_i, "wv": wv_t, "wq": wq_t, "qt": qt_i,
                "wo": wo_t, "bvt": bv_t, "bqt": bq_t, "bo": bo_r,
                "mf": mf_i, "madd": madd_i, "id4": np.eye(JL, dtype=np.float32),
            }
        )
    return N, in_maps


_NC_CACHE = {}


def get_graph(N, mm="fp8"):
    key = (N, mm)
    if key not in _NC_CACHE:
        _NC_CACHE[key] = build_graph(N, mm)
    return _NC_CACHE[key]


def run(inputs, trace=False, mm="fp8"):
    N, in_maps = prep_inputs(**inputs, mm=mm)
    nc = get_graph(N, mm)
    res = bass_utils.run_bass_kernel_spmd(
        nc, in_maps, core_ids=list(range(NCORES)), trace=trace
    )
    out = np.concatenate([res.results[i]["out"] for i in range(NCORES)], axis=0)
    return out.astype(np.float32), res


def kernel(**inputs):
    out, _ = run(inputs, trace=False)
    return out


# revision 4
# speedup vs baseline: 2.0754x; 1.3699x over previous
"""Trainium2 Bass kernel for the masked-softmax attention module.

Computation (per batch row b):
    m      = lrelu(values[b] @ Wv.T + bv) + lrelu(query[b] @ Wq.T + bq)   [L, A]
    logit  = lrelu(tanh(m) @ Wo.T + bo)                                    [L]
    logit  = where(mask[b] == 0, -1e-9, logit)
    prob   = softmax(logit)
    out[b] = prob @ values[b]                                              [D]

Sparsity: positions with mask==0 get logit = -1e-9, so their softmax
weight is exactly exp(-1e-9) == 1.0f regardless of the expensive
pipeline. Host-side we PERMUTE each batch's L dim so mask==1 positions
come first (n1 of them), and only compute the m/tanh/Wo pipeline for
the first N >= max_b(n1) positions. Logits at [n1, N) are masked to
-1e-9 (exactly as the reference masks them) and [N, L) are memset to 0
(exp(0) == exp(-1e-9) == 1.0f). The softmax + out GEMM then run over
the full permuted L — numerically identical to the dense reference.

Main GEMM runs in fp8 (e4m3): values cast directly (absmax ~5.4 << 240),
Wv pre-scaled by 2^10 so its entries are normal-range; the 2^-10 unscale
is folded into the lrelu ACT's scale input (exact, power of two).
DoubleRow perf mode processes 2 k-tiles per matmul at 0.5 cycles/row.

Sharding: data-parallel over batch, 4 batches per core on 8 NeuronCores.
"""

import os
import sys

if "/opt/trn_rl_repo" not in sys.path:
    sys.path.insert(0, "/opt/trn_rl_repo")

import numpy as np
import ml_dtypes

from contextlib import ExitStack

import concourse.bass as bass
import concourse.tile as tile
from concourse import bacc, mybir
from concourse import bass_utils

BF = ml_dtypes.bfloat16
E4 = ml_dtypes.float8_e4m3
F32d = mybir.dt.float32
BF16d = mybir.dt.bfloat16
FP8d = mybir.dt.float8e4
AF = mybir.ActivationFunctionType
DR = mybir.MatmulPerfMode.DoubleRow

NCORES = 8
B, L, D, A = 32, 1024, 2048, 2048
BL = B // NCORES          # batches per core
KD = D // 128             # d tiles
KA = A // 128             # a tiles
JL = L // 128             # l tiles
ALPHA = 0.01              # leaky relu slope
WV_SCALE = 1024.0         # host premultiplier on Wv for fp8 dynamic range


def build_graph(N, mm="fp8"):
    """Build the per-core Bass graph (identical on all cores).

    N: padded count of computed positions per batch (even).
    mm: "fp8" (DoubleRow e4m3 main GEMM) or "bf16".
    """
    nc = bacc.Bacc("TRN2", target_bir_lowering=False, debug=False)
    NL1 = N // 2
    chunks = [(0, NL1), (NL1, NL1)]
    vdt, vnp_dt = (FP8d, E4) if mm == "fp8" else (BF16d, BF)

    vt = nc.dram_tensor("vt", [BL, 128, KD, N], vdt, kind="ExternalInput")
    vn = nc.dram_tensor("vn", [BL, 128, JL, D], BF16d, kind="ExternalInput")
    wv = nc.dram_tensor("wv", [128, KA, KD, 128], vdt, kind="ExternalInput")
    wq = nc.dram_tensor("wq", [128, KA, KD, 128], BF16d, kind="ExternalInput")
    qt = nc.dram_tensor("qt", [128, KD, BL], BF16d, kind="ExternalInput")
    wo = nc.dram_tensor("wo", [128, KA], BF16d, kind="ExternalInput")
    bvt = nc.dram_tensor("bvt", [128, KA], F32d, kind="ExternalInput")
    bqt = nc.dram_tensor("bqt", [128, KA], F32d, kind="ExternalInput")
    bo = nc.dram_tensor("bo", [1, 1], F32d, kind="ExternalInput")
    mf = nc.dram_tensor("mf", [1, BL * N], F32d, kind="ExternalInput")
    madd = nc.dram_tensor("madd", [1, BL * N], F32d, kind="ExternalInput")
    id4d = nc.dram_tensor("id4", [JL, JL], F32d, kind="ExternalInput")
    out = nc.dram_tensor("out", [BL, D], F32d, kind="ExternalOutput")

    lr_scale = 1.0 / WV_SCALE if mm == "fp8" else 1.0

    with tile.TileContext(nc) as tc, ExitStack() as ctx:
        const = ctx.enter_context(tc.tile_pool(name="const", bufs=1))
        wvp = ctx.enter_context(tc.tile_pool(name="wvp", bufs=1))

        # One FIFO HWDGE ring (sync) carries all latency-ordered loads in exact
        # need-order; only bulk vn (needed a full batch later) rides the
        # scalar ring in parallel.
        qts_sb = const.tile([128, KD, BL], BF16d)
        nc.sync.dma_start(qts_sb[:], qt.ap()[:])
        bq_sb = const.tile([128, KA], F32d)
        nc.sync.dma_start(bq_sb[:], bqt.ap()[:])
        id8 = const.tile([JL, JL], F32d)
        nc.scalar.dma_start(id8[:], id4d.ap()[:])
        qp_sb = const.tile([128, KA, BL], F32d)
        # allocated here, DMA'd after the first vt chunk (see main loop)
        wo_sb = const.tile([128, KA], BF16d)
        bv_sb = const.tile([128, KA], F32d)
        bo_sb = const.tile([1, 1], F32d)
        mf_sb = const.tile([1, BL * N], F32d)
        madd_sb = const.tile([1, BL * N], F32d)

        # wv is laid out a-tile-major: GEMM group j only needs its own chunk,
        # so chunks stream just-in-time, interleaved with wq below.
        wv_sb = wvp.tile([128, KA, KD, 128], vdt)
        wv_loaded = set()

        def fetch_wv(j):
            if j < KA and j not in wv_loaded:
                nc.sync.dma_start(wv_sb[:, j, :, :], wv.ap()[:, j, :, :])
                wv_loaded.add(j)

        # q-projection is interleaved into batch 0 / chunk 0 of the main loop
        # (one group per GEMM group) so its wq DMA demand spreads out and the
        # PE never sits idle waiting for the projection phase.
        wqp = ctx.enter_context(tc.tile_pool(name="wqp", bufs=3))
        psqp = ctx.enter_context(tc.tile_pool(name="psq", bufs=1, space="PSUM"))
        wq_tiles = {}

        def fetch_wq(t):
            if t < KA and t not in wq_tiles:
                wq_t = wqp.tile([128, KD, 128], BF16d)
                nc.sync.dma_start(wq_t[:], wq.ap()[:, t, :, :])
                wq_tiles[t] = wq_t

        def qproj_group(t):
            wq_t = wq_tiles.pop(t)
            psq = psqp.tile([128, BL], F32d)
            for k in range(KD):
                nc.tensor.matmul(
                    psq[:], lhsT=wq_t[:, k, :], rhs=qts_sb[:, k, :],
                    start=(k == 0), stop=(k == KD - 1),
                )
            nc.scalar.activation(
                qp_sb[:, t, :], psq[:], AF.Lrelu, bias=bq_sb[:, t : t + 1],
                alpha=ALPHA,
            )

        # PE warmup: dummy matmuls on zeroed tiles while the first DMAs land,
        # so the HAM clock gate is released before real work starts.
        wu_l = const.tile([128, 128], BF16d)
        nc.vector.memset(wu_l[:], 0.0)
        wu_ps = psqp.tile([128, 128], F32d, tag="psq")
        for i in range(32):
            nc.tensor.matmul(wu_ps[:], lhsT=wu_l[:], rhs=wu_l[:], start=(i == 0), stop=(i == 31))

        # ---- main loop ----
        vtp = ctx.enter_context(tc.tile_pool(name="vtp", bufs=2))
        vnp = ctx.enter_context(tc.tile_pool(name="vnp", bufs=1))
        s1p = ctx.enter_context(tc.tile_pool(name="s1p", bufs=2))
        thp = ctx.enter_context(tc.tile_pool(name="thp", bufs=3))
        smp = ctx.enter_context(tc.tile_pool(name="smp", bufs=1))
        outp = ctx.enter_context(tc.tile_pool(name="outp", bufs=1))
        psm = ctx.enter_context(tc.tile_pool(name="psm", bufs=2, space="PSUM"))
        psl = ctx.enter_context(tc.tile_pool(name="psl", bufs=2, space="PSUM"))
        pst = ctx.enter_context(tc.tile_pool(name="pst", bufs=1, space="PSUM"))
        pso = ctx.enter_context(tc.tile_pool(name="pso", bufs=2, space="PSUM"))

        for b in range(BL):
            vn_b = None
            logit_sb = smp.tile([1, L], F32d, tag="logit", bufs=2)
            # tail [N, L) is never computed: weight must be exp(-1e-9) == 1.0f,
            # which equals exp(0), so zero-fill suffices.
            nc.vector.memset(logit_sb[:, N:L], 0.0)
            vt_b = vtp.tile([128, KD, N], vdt)
            nc.sync.dma_start(vt_b[:], vt.ap()[b, :, :, :])
            for ci, (off, nl) in enumerate(chunks):
                if ci == len(chunks) - 1:
                    # natural-orientation values load, deferred past the
                    # DMA-congested first chunk (only needed at batch end)
                    vn_b = vnp.tile([128, JL, D], BF16d)
                    nc.scalar.dma_start(vn_b[:], vn.ap()[b, :, :, :])
                if b == 0 and ci == 0:
                    fetch_wq(0)
                    fetch_wv(0)
                    fetch_wq(1)
                    nc.scalar.dma_start(wo_sb[:], wo.ap()[:])
                    nc.scalar.dma_start(bv_sb[:], bvt.ap()[:])
                    nc.scalar.dma_start(bo_sb[:], bo.ap()[:])
                    nc.scalar.dma_start(mf_sb[:], mf.ap()[:])
                    nc.scalar.dma_start(madd_sb[:], madd.ap()[:])
                ps_l = psl.tile([1, 512], F32d)
                pending = None
                for j in range(KA):
                    if b == 0 and ci == 0:
                        qproj_group(j)
                        fetch_wq(j + 2)
                        fetch_wv(j + 1)
                        fetch_wv(j + 2)
                    ps_m = psm.tile([128, 512], F32d)
                    if mm == "fp8":
                        for q in range(KD // 2):
                            nc.tensor.matmul(
                                ps_m[:, :nl],
                                lhsT=wv_sb[:, j, 2 * q : 2 * q + 2, :],
                                rhs=vt_b[:, 2 * q : 2 * q + 2, off : off + nl],
                                start=(q == 0),
                                stop=(q == KD // 2 - 1),
                                perf_mode=DR,
                            )
                    else:
                        for k in range(KD):
                            nc.tensor.matmul(
                                ps_m[:, :nl],
                                lhsT=wv_sb[:, j, k, :],
                                rhs=vt_b[:, k, off : off + nl],
                                start=(k == 0),
                                stop=(k == KD - 1),
                            )
                    s1 = s1p.tile([128, 512], F32d)
                    nc.scalar.activation(
                        s1[:, :nl], ps_m[:, :nl], AF.Lrelu,
                        bias=bv_sb[:, j : j + 1], scale=lr_scale, alpha=ALPHA,
                    )
                    th = thp.tile([128, 512], BF16d)
                    nc.scalar.activation(
                        th[:, :nl], s1[:, :nl], AF.Tanh, bias=qp_sb[:, j, b : b + 1]
                    )
                    if pending is not None:
                        pj, pth = pending
                        nc.tensor.matmul(
                            ps_l[:, :nl], lhsT=wo_sb[:, pj : pj + 1], rhs=pth[:, :nl],
                            start=(pj == 0), stop=False,
                        )
                    pending = (j, th)
                pj, pth = pending
                nc.tensor.matmul(
                    ps_l[:, :nl], lhsT=wo_sb[:, pj : pj + 1], rhs=pth[:, :nl],
                    start=False, stop=True,
                )
                lsl = logit_sb[:, off : off + nl]
                nc.scalar.activation(
                    lsl, ps_l[:, :nl], AF.Lrelu, bias=bo_sb[0:1, 0:1], alpha=ALPHA
                )
                # apply the mask per chunk (off the end-of-batch critical path)
                o = b * N + off
                nc.vector.tensor_mul(lsl, lsl, mf_sb[:, o : o + nl])
                nc.vector.tensor_add(lsl, lsl, madd_sb[:, o : o + nl])

            # ---- softmax (no max-subtract: logits are lrelu-bounded, exp stays
            # well inside f32 range; identical ratios to the reference) ----
            p_f = smp.tile([1, L], F32d, tag="pf", bufs=2)
            ssum = smp.tile([1, 1], F32d, tag="ss", bufs=2)
            nc.scalar.activation(p_f[:], logit_sb[:], AF.Exp, accum_out=ssum[:])
            rs = smp.tile([1, 1], F32d, tag="rs", bufs=2)
            nc.vector.reciprocal(rs[:], ssum[:])
            # cross-partition reshape [1, L] -> [JL, 128] (tiny DMA), then one
            # PE transpose to [128, JL]
            p8 = smp.tile([JL, 128], F32d, tag="p8", bufs=2)
            nc.scalar.dma_start(p8[:], p_f[:])
            ps_t = pst.tile([128, JL], F32d)
            nc.tensor.transpose(ps_t[:], p8[:], id8[:])
            pT = smp.tile([128, JL], BF16d, tag="pT", bufs=2)
            nc.vector.tensor_copy(pT[:], ps_t[:])

            # ---- out[b] = (p @ values) / sum  (1/sum folded into the copy) ----
            out_sb = outp.tile([1, D], F32d, bufs=2)
            for dc in range(4):
                ps_o = pso.tile([1, 512], F32d)
                for t in range(JL):
                    nc.tensor.matmul(
                        ps_o[:], lhsT=pT[:, t : t + 1],
                        rhs=vn_b[:, t, 512 * dc : 512 * dc + 512],
                        start=(t == 0), stop=(t == JL - 1),
                    )
                osl = out_sb[:, 512 * dc : 512 * dc + 512]
                nc.vector.tensor_scalar_mul(osl, ps_o[:], rs[0:1, 0:1])
                nc.sync.dma_start(out.ap()[b : b + 1, 512 * dc : 512 * dc + 512], osl)

    nc.compile()
    return nc


def pad_n(max_n1):
    """Computed-position count: even split into two equal chunks, each a
    multiple of 4 and >= 128 so the PE never stalls on LDWEIGHTS."""
    half = max(128, (max_n1 + 1) // 2)
    half = (half + 3) // 4 * 4
    return min(2 * half, L)


def prep_inputs(query, values, mask, Wq, bq, Wv, bv, Wo, bo, mm="fp8"):
    """Host-side shard + layout prep. Returns (N, list of 8 in_maps)."""
    mask = np.asarray(mask)
    n1s = (mask != 0).sum(axis=1)
    N = pad_n(int(n1s.max()))

    Wv32 = np.ascontiguousarray(Wv, np.float32)
    Wq32 = np.ascontiguousarray(Wq, np.float32)
    # wv[p, j, k, i] = Wv[128j+i, 128k+p]  (WvT, a-tile-major chunks)
    wv_t = np.ascontiguousarray(
        Wv32.reshape(KA, 128, KD, 128).transpose(3, 0, 2, 1)
    )
    if mm == "fp8":
        wv_t = (wv_t * np.float32(WV_SCALE)).astype(E4)
    else:
        wv_t = wv_t.astype(BF)
    # wq[p, t, k, i] = Wq[128t+i, 128k+p]  (WqT, a-tile-major chunks)
    wq_t = np.ascontiguousarray(
        Wq32.reshape(KA, 128, KD, 128).transpose(3, 0, 2, 1)
    ).astype(BF)
    wo_t = np.ascontiguousarray(Wo.reshape(KA, 128).T).astype(BF)
    bv_t = np.ascontiguousarray(bv.reshape(KA, 128).T).astype(np.float32)
    bq_t = np.ascontiguousarray(bq.reshape(KA, 128).T).astype(np.float32)
    bo_r = np.asarray(bo, np.float32).reshape(1, 1)

    in_maps = []
    for i in range(NCORES):
        sl = slice(BL * i, BL * (i + 1))
        v = np.asarray(values[sl], np.float32)
        m = np.asarray(mask[sl])
        # permute each batch's L dim: mask!=0 positions first
        vp = np.empty_like(v)
        mp = np.empty_like(m)
        for bb in range(BL):
            perm = np.concatenate(
                [np.flatnonzero(m[bb] != 0), np.flatnonzero(m[bb] == 0)]
            )
            vp[bb] = v[bb, perm]
            mp[bb] = m[bb, perm]
        # vt[b, p, k, l] = vp[b, l, 128k+p] for l < N
        vt_i = np.ascontiguousarray(
            vp[:, :N, :].reshape(BL, N, KD, 128).transpose(0, 3, 2, 1)
        )
        vt_i = vt_i.astype(E4) if mm == "fp8" else vt_i.astype(BF)
        # vn[b, p, j, d] = vp[b, 128j+p, d]
        vn_i = np.ascontiguousarray(
            vp.reshape(BL, JL, 128, D).transpose(0, 2, 1, 3)
        ).astype(BF)
        # qt[p, k, b] = query[b, 128k+p]
        qt_i = np.ascontiguousarray(
            np.asarray(query[sl], np.float32).T.reshape(KD, 128, BL).transpose(1, 0, 2)
        ).astype(BF)
        mf_i = (mp[:, :N] != 0).astype(np.float32).reshape(1, BL * N)
        madd_i = ((mp[:, :N] == 0).astype(np.float32) * np.float32(-1e-9)).reshape(
            1, BL * N
        )
        in_maps.append(
            {
                "vt": vt_i, "vn": vn_i, "wv": wv_t, "wq": wq_t, "qt": qt_i,
                "wo": wo_t, "bvt": bv_t, "bqt": bq_t, "bo": bo_r,
                "mf": mf_i, "madd": madd_i, "id4": np.eye(JL, dtype=np.float32),
            }
        )
    return N, in_maps


_NC_CACHE = {}


def get_graph(N, mm="fp8"):
    key = (N, mm)
    if key not in _NC_CACHE:
        _NC_CACHE[key] = build_graph(N, mm)
    return _NC_CACHE[key]


def run(inputs, trace=False, mm="fp8"):
    N, in_maps = prep_inputs(**inputs, mm=mm)
    nc = get_graph(N, mm)
    res = bass_utils.run_bass_kernel_spmd(
        nc, in_maps, core_ids=list(range(NCORES)), trace=trace
    )
    out = np.concatenate([res.results[i]["out"] for i in range(NCORES)], axis=0)
    return out.astype(np.float32), res


def kernel(**inputs):
    out, _ = run(inputs, trace=False)
    return out
